# revision 30
# baseline (speedup 1.0000x reference)
"""CLUB loss kernel for 8x TRN2 NeuronCores.

Math: the reference computes, per sample b (L=512 positions, D=64 dims):
  mu     = MLP_mu(x);  logvar = tanh(MLP_lv(x));  iv = exp(-logvar)
  positive[d,l] = -(mu - y)^2 * 0.5 * iv
  negative[d,l] = -mean_j (y[d,j] - mu[d,l])^2 * 0.5 * iv
  loss = mean over (b,l) of sum_d (positive - negative)

The pairwise LxL mean collapses via moments of y over positions:
  mean_j (y_j - mu)^2 = Ey2 - 2*mu*Ey + mu^2
so with yd2 = 2*(y - Ey), ysq = y^2, mu = mu_nb + b2:
  loss = -0.5/(B*L) * sum_{b,d,l} [ ((ysq - Ey2) - mu*yd2) * iv ]
       = -0.5/(B*L) * [ sum((ysq - Ey2)*iv) - sum(mu * (yd2*iv)) ]
The two sums accumulate independently (finA on GpSimd, finB on DVE) and are
collapsed on-chip with a ones-vector matmul so the store is one 16-byte
single-packet DMA (a [64,1] store costs 64 tiny descriptors + 16 lazy
semaphore increments, ~5us observed).

Sharding: data-parallel over batch B=8, one sample per core; host does the
tiny final combine.

Precision/speed choices (fp32 matmul is 4 cycles/col on the PE):
- layer-1 matmul operands are float32r (fp32 bits, single-pass PE mode);
- relu outputs h and the W2 weights are bf16 (also single-pass; adds ~nothing
  to the fp32r error, measured);
- y, moments and all elementwise math stay fp32.
- A run of dummy bf16 matmuls during the input-DMA wait keeps the PE activity
  monitor from dropping the array to its cold 1.2 GHz clock.
"""

import sys

if "/opt/trn_rl_repo" not in sys.path:
    sys.path.insert(0, "/opt/trn_rl_repo")

import numpy as np

B, L = 8, 512
XD, YD, H = 192, 64, 128
NCORES = 8
HC = L // 2
YBC = 516  # y (512) + b2mu, b2lv, pad, pad

_CACHE: dict = {}


def build_nc(debug: bool = False, warmup_mms: int = 12):
    import concourse.bass as bass
    import concourse.bacc as bacc
    import concourse.tile as tile
    from concourse import mybir

    f32 = mybir.dt.float32
    f32r = mybir.dt.float32r
    bf16 = mybir.dt.bfloat16
    AF = mybir.ActivationFunctionType
    OP = mybir.AluOpType

    nc = bacc.Bacc("TRN2", target_bir_lowering=False, debug=debug)

    w1_d = nc.dram_tensor("w1", [128, 512], f32r, kind="ExternalInput")
    w2_d = nc.dram_tensor("w2", [128, 128], bf16, kind="ExternalInput")
    xa_d = nc.dram_tensor("xa", [128, L], f32r, kind="ExternalInput")
    xb_d = nc.dram_tensor("xb", [64, L], f32r, kind="ExternalInput")
    yb_d = nc.dram_tensor("yb", [64, YBC], f32, kind="ExternalInput")
    b1_d = nc.dram_tensor("b1", [128, 2], f32, kind="ExternalInput")
    acc_d = nc.dram_tensor("acc", [4, 2], f32, kind="ExternalOutput")

    with tile.TileContext(nc) as tc:
        with (
            tc.tile_pool(name="sb", bufs=1) as sb,
            tc.tile_pool(name="ps", bufs=1, space=bass.MemorySpace.PSUM) as ps,
        ):
            ones = sb.tile([64, 1], f32, tag="ones")
            nc.gpsimd.memset(ones, 1.0)
            junk = sb.tile([64, L], bf16, tag="junk")
            nc.gpsimd.memset(junk, 0.0)

            # input DMAs spread over sync/scalar HWDGE rings + SWDGE
            w1t = sb.tile([128, 512], f32r, tag="w1t")
            nc.sync.dma_start(out=w1t, in_=w1_d[:, :])
            w2t = sb.tile([128, 128], bf16, tag="w2t")
            nc.sync.dma_start(out=w2t, in_=w2_d[:, :])
            xat = sb.tile([128, L], f32r, tag="xat")
            nc.scalar.dma_start(out=xat, in_=xa_d[:, :])
            xbr = sb.tile([128, L], f32r, tag="xbr")
            nc.scalar.dma_start(out=xbr[64:128, :], in_=xb_d[:, :])
            ybt = sb.tile([64, YBC], f32, tag="ybt")
            nc.gpsimd.dma_start(out=ybt, in_=yb_d[:, :])
            b1t = sb.tile([128, 2], f32, tag="b1t")
            nc.gpsimd.dma_start(out=b1t, in_=b1_d[:, :])

            # PE warm-up while DMAs are in flight
            warm_ps = ps.tile([1, L], f32, tag="warm")
            for _ in range(warmup_mms):
                nc.tensor.matmul(warm_ps, junk[:, 0:1], junk[:, :], start=True, stop=True)

            w1muT_a = w1t[:, 0:128]
            w1lvT_a = w1t[:, 128:256]
            w1muT_b = w1t[64:128, 256:384]
            w1lvT_b = w1t[64:128, 384:512]
            w2muT = w2t[:, 0:64]
            w2lvT = w2t[:, 64:128]
            b1mu = b1t[:, 0:1]
            b1lv = b1t[:, 1:2]
            xa = xat[:, :]
            xb = xbr[64:128, :]
            y = ybt[:, 0:512]
            b2mu = ybt[:, 512:513]
            b2lv = ybt[:, 513:514]

            # --- moments of y (DVE, early — overlaps the DMA/matmul wait) ---
            sums = sb.tile([64, 2], f32, tag="sums")
            nc.vector.reduce_sum(out=sums[:, 0:1], in_=y, axis=mybir.AxisListType.X)
            ysq = sb.tile([64, L], f32, tag="ysq")
            nc.vector.scalar_tensor_tensor(
                out=ysq, in0=y, scalar=1.0, in1=y,
                op0=OP.mult, op1=OP.mult, accum_out=sums[:, 1:2],
            )
            eyb = sb.tile([64, 2], f32, tag="eyb")
            nc.vector.tensor_scalar_mul(out=eyb, in0=sums, scalar1=1.0 / L)
            ey = eyb[:, 0:1]
            ey2 = eyb[:, 1:2]
            yd2 = sb.tile([64, L], f32, tag="yd2")
            nc.vector.tensor_scalar(
                out=yd2, in0=y, scalar1=ey, scalar2=2.0, op0=OP.subtract, op1=OP.mult
            )

            # --- layer 1 (f32r, full-width) ---
            h_lv = ps.tile([128, L], f32, tag="hlv")
            h_mu = ps.tile([128, L], f32, tag="hmu")
            nc.tensor.matmul(h_lv, w1lvT_a, xa, start=True, stop=False)
            nc.tensor.matmul(h_mu, w1muT_a, xa, start=True, stop=False)
            nc.tensor.matmul(h_lv, w1lvT_b, xb, start=False, stop=True)
            nc.tensor.matmul(h_mu, w1muT_b, xb, start=False, stop=True)

            # --- layer 2 (bf16) + tail, chunked over L halves ---
            h_lv_s = sb.tile([128, L], bf16, tag="hlvs")
            h_mu_s = sb.tile([128, L], bf16, tag="hmus")
            acc4 = sb.tile([64, 4], f32, tag="acc4")
            lv_nbs = []
            mu_nbs = []
            for c in range(2):
                cs = slice(c * HC, (c + 1) * HC)
                # lv path first: it is one ACT stage deeper (tanh+exp)
                nc.scalar.activation(
                    out=h_lv_s[:, cs], in_=h_lv[:, cs], func=AF.Relu, bias=b1lv, scale=1.0
                )
                lv_nb = ps.tile([64, HC], f32, tag="lvnb")
                nc.tensor.matmul(lv_nb, w2lvT, h_lv_s[:, cs], start=True, stop=True)
                lv_nbs.append(lv_nb)
                # mu relu on DVE to keep ACT free for tanh/exp
                nc.vector.tensor_scalar(
                    out=h_mu_s[:, cs], in0=h_mu[:, cs], scalar1=b1mu, scalar2=0.0,
                    op0=OP.add, op1=OP.max,
                )
                mu_nb = ps.tile([64, HC], f32, tag="munb")
                nc.tensor.matmul(mu_nb, w2muT, h_mu_s[:, cs], start=True, stop=True)
                mu_nbs.append(mu_nb)

            # sum(iv) per half comes free from exp's accumulator; the ey2
            # cross-term collapses later via one dot-product matmul:
            #   sum((ysq-ey2)*iv) = sum(ysq*iv) - sum_d ey2[d]*sum_l iv[d,l]
            siv = sb.tile([64, 2], f32, tag="siv")
            for c in range(2):
                cs = slice(c * HC, (c + 1) * HC)
                t1 = sb.tile([64, HC], f32, tag="t1")
                nc.scalar.activation(
                    out=t1, in_=lv_nbs[c], func=AF.Tanh, bias=b2lv, scale=1.0
                )
                iv = sb.tile([64, HC], f32, tag="iv")
                nc.scalar.activation(
                    out=iv, in_=t1, func=AF.Exp, scale=-1.0,
                    accum_out=siv[:, c : c + 1],
                )
                # z = yd2 * iv on GpSimd (otherwise idle)
                z = sb.tile([64, HC], f32, tag="z")
                nc.gpsimd.tensor_mul(z, yd2[:, cs], iv)
                # finA1 = sum(ysq * iv)
                scrA = sb.tile([64, HC], f32, tag="scrA")
                nc.vector.scalar_tensor_tensor(
                    out=scrA, in0=ysq[:, cs], scalar=1.0, in1=iv,
                    op0=OP.mult, op1=OP.mult, accum_out=acc4[:, 2 * c : 2 * c + 1],
                )
                # finB = sum((mu_nb + b2mu) * z)
                scrB = sb.tile([64, HC], f32, tag="scrB")
                nc.vector.scalar_tensor_tensor(
                    out=scrB, in0=mu_nbs[c], scalar=b2mu, in1=z,
                    op0=OP.add, op1=OP.mult, accum_out=acc4[:, 2 * c + 1 : 2 * c + 2],
                )

            acc_ps = ps.tile([4, 1], f32, tag="accps")
            nc.tensor.matmul(acc_ps, acc4, ones, start=True, stop=True)
            civ_ps = ps.tile([2, 1], f32, tag="civps")
            nc.tensor.matmul(civ_ps, siv, ey2, start=True, stop=True)
            acc_sb = sb.tile([4, 2], f32, tag="accsb")
            nc.scalar.copy(acc_sb[:, 0:1], acc_ps)
            nc.scalar.copy(acc_sb[0:2, 1:2], civ_ps)
            nc.sync.dma_start(out=acc_d[:, :], in_=acc_sb, single_packet=True)

    nc.compile()
    return nc


def pack_inputs(inputs: dict) -> list[dict]:
    import ml_dtypes

    x = np.ascontiguousarray(np.asarray(inputs["x_samples"], dtype=np.float32))
    y = np.ascontiguousarray(np.asarray(inputs["y_samples"], dtype=np.float32))
    mu_W1 = np.asarray(inputs["mu_W1"], dtype=np.float32)
    mu_b1 = np.asarray(inputs["mu_b1"], dtype=np.float32)
    mu_W2 = np.asarray(inputs["mu_W2"], dtype=np.float32)
    mu_b2 = np.asarray(inputs["mu_b2"], dtype=np.float32)
    lv_W1 = np.asarray(inputs["lv_W1"], dtype=np.float32)
    lv_b1 = np.asarray(inputs["lv_b1"], dtype=np.float32)
    lv_W2 = np.asarray(inputs["lv_W2"], dtype=np.float32)
    lv_b2 = np.asarray(inputs["lv_b2"], dtype=np.float32)

    w1 = np.zeros((128, 512), np.float32)
    w1muT = mu_W1.T  # [192, 128]
    w1lvT = lv_W1.T
    w1[:, 0:128] = w1muT[0:128]
    w1[:, 128:256] = w1lvT[0:128]
    w1[64:128, 256:384] = w1muT[128:192]
    w1[64:128, 384:512] = w1lvT[128:192]
    w2 = np.concatenate([mu_W2.T, lv_W2.T], axis=1).astype(ml_dtypes.bfloat16)
    b1 = np.ascontiguousarray(np.stack([mu_b1, lv_b1], axis=1))  # [128, 2]

    in_maps = []
    for b in range(NCORES):
        yb = np.zeros((64, YBC), np.float32)
        yb[:, 0:512] = y[b]
        yb[:, 512] = mu_b2
        yb[:, 513] = lv_b2
        in_maps.append(
            {
                "w1": w1,
                "w2": np.ascontiguousarray(w2),
                "xa": np.ascontiguousarray(x[b, 0:128]),
                "xb": np.ascontiguousarray(x[b, 128:192]),
                "yb": yb,
                "b1": b1,
            }
        )
    return in_maps


def kernel(**inputs) -> np.ndarray:
    from concourse.bass_utils import run_bass_kernel_spmd

    if "nc" not in _CACHE:
        _CACHE["nc"] = build_nc(debug=False)
    nc = _CACHE["nc"]

    in_maps = pack_inputs(inputs)
    res = run_bass_kernel_spmd(nc, in_maps, core_ids=list(range(NCORES)))
    tot = 0.0
    for r in res.results:
        a = r["acc"].astype(np.float64)  # [4, 2]
        tot += (a[0, 0] + a[2, 0]) - (a[1, 0] + a[3, 0]) - (a[0, 1] + a[1, 1])
    loss = -0.5 * tot / (B * L)
    return np.array(loss, dtype=np.float32)


# revision 33
# speedup vs baseline: 1.0796x; 1.0796x over previous
"""CLUB loss kernel for 8x TRN2 NeuronCores.

Math: the reference computes, per sample b (L=512 positions, D=64 dims):
  mu     = MLP_mu(x);  logvar = tanh(MLP_lv(x));  iv = exp(-logvar)
  positive[d,l] = -(mu - y)^2 * 0.5 * iv
  negative[d,l] = -mean_j (y[d,j] - mu[d,l])^2 * 0.5 * iv
  loss = mean over (b,l) of sum_d (positive - negative)

The pairwise LxL mean collapses via moments of y over positions:
  mean_j (y_j - mu)^2 = Ey2 - 2*mu*Ey + mu^2
so with yd2 = 2*(y - Ey), ysq = y^2, mu = mu_nb + b2:
  loss = -0.5/(B*L) * sum_{b,d,l} [ ((ysq - Ey2) - mu*yd2) * iv ]
       = -0.5/(B*L) * [ sum((ysq - Ey2)*iv) - sum(mu * (yd2*iv)) ]
The two sums accumulate independently (finA on GpSimd, finB on DVE) and are
collapsed on-chip with a ones-vector matmul so the store is one 16-byte
single-packet DMA (a [64,1] store costs 64 tiny descriptors + 16 lazy
semaphore increments, ~5us observed).

Sharding: data-parallel over batch B=8, one sample per core; host does the
tiny final combine.

Precision/speed choices (fp32 matmul is 4 cycles/col on the PE):
- layer-1 matmul operands are float32r (fp32 bits, single-pass PE mode);
- relu outputs h and the W2 weights are bf16 (also single-pass; adds ~nothing
  to the fp32r error, measured);
- y, moments and all elementwise math stay fp32.
- A run of dummy bf16 matmuls during the input-DMA wait keeps the PE activity
  monitor from dropping the array to its cold 1.2 GHz clock.
"""

import sys

if "/opt/trn_rl_repo" not in sys.path:
    sys.path.insert(0, "/opt/trn_rl_repo")

import numpy as np

B, L = 8, 512
XD, YD, H = 192, 64, 128
NCORES = 8
HC = L // 2
YBC = 516  # y (512) + b2mu, b2lv, pad, pad

_CACHE: dict = {}


def build_nc(debug: bool = False, warmup_mms: int = 0):
    import concourse.bass as bass
    import concourse.bacc as bacc
    import concourse.tile as tile
    from concourse import mybir

    f32 = mybir.dt.float32
    f32r = mybir.dt.float32r
    bf16 = mybir.dt.bfloat16
    AF = mybir.ActivationFunctionType
    OP = mybir.AluOpType

    nc = bacc.Bacc("TRN2", target_bir_lowering=False, debug=debug)

    w1_d = nc.dram_tensor("w1", [128, 512], f32r, kind="ExternalInput")
    w2_d = nc.dram_tensor("w2", [128, 128], bf16, kind="ExternalInput")
    xa_d = nc.dram_tensor("xa", [128, L], f32r, kind="ExternalInput")
    xb_d = nc.dram_tensor("xb", [64, L], f32r, kind="ExternalInput")
    yb_d = nc.dram_tensor("yb", [64, YBC], f32, kind="ExternalInput")
    b1_d = nc.dram_tensor("b1", [128, 2], f32, kind="ExternalInput")
    acc_d = nc.dram_tensor("acc", [4, 2], f32, kind="ExternalOutput")

    with tile.TileContext(nc) as tc:
        with (
            tc.tile_pool(name="sb", bufs=1) as sb,
            tc.tile_pool(name="ps", bufs=1, space=bass.MemorySpace.PSUM) as ps,
        ):
            ones = sb.tile([64, 1], f32, tag="ones")
            nc.gpsimd.memset(ones, 1.0)

            # input DMAs spread over sync/scalar HWDGE rings + SWDGE
            # (the scalar ring is empirically faster, so it gets w1 — the
            # first matmul gate)
            w1t = sb.tile([128, 512], f32r, tag="w1t")
            nc.scalar.dma_start(out=w1t, in_=w1_d[:, :])
            xbr = sb.tile([128, L], f32r, tag="xbr")
            nc.scalar.dma_start(out=xbr[64:128, :], in_=xb_d[:, :])
            xat = sb.tile([128, L], f32r, tag="xat")
            nc.sync.dma_start(out=xat, in_=xa_d[:, :])
            w2t = sb.tile([128, 128], bf16, tag="w2t")
            nc.sync.dma_start(out=w2t, in_=w2_d[:, :])
            ybt = sb.tile([64, YBC], f32, tag="ybt")
            nc.gpsimd.dma_start(out=ybt, in_=yb_d[:, :])
            b1t = sb.tile([128, 2], f32, tag="b1t")
            nc.gpsimd.dma_start(out=b1t, in_=b1_d[:, :])

            if warmup_mms:
                junk = sb.tile([64, L], bf16, tag="junk")
                nc.gpsimd.memset(junk, 0.0)
                warm_ps = ps.tile([1, L], f32, tag="warm")
                for _ in range(warmup_mms):
                    nc.tensor.matmul(
                        warm_ps, junk[:, 0:1], junk[:, :], start=True, stop=True
                    )

            w1muT_a = w1t[:, 0:128]
            w1lvT_a = w1t[:, 128:256]
            w1muT_b = w1t[64:128, 256:384]
            w1lvT_b = w1t[64:128, 384:512]
            w2muT = w2t[:, 0:64]
            w2lvT = w2t[:, 64:128]
            b1mu = b1t[:, 0:1]
            b1lv = b1t[:, 1:2]
            xa = xat[:, :]
            xb = xbr[64:128, :]
            y = ybt[:, 0:512]
            b2mu = ybt[:, 512:513]
            b2lv = ybt[:, 513:514]

            # --- moments of y (DVE, early — overlaps the DMA/matmul wait) ---
            sums = sb.tile([64, 2], f32, tag="sums")
            nc.vector.reduce_sum(out=sums[:, 0:1], in_=y, axis=mybir.AxisListType.X)
            ysq = sb.tile([64, L], f32, tag="ysq")
            nc.vector.scalar_tensor_tensor(
                out=ysq, in0=y, scalar=1.0, in1=y,
                op0=OP.mult, op1=OP.mult, accum_out=sums[:, 1:2],
            )
            eyb = sb.tile([64, 2], f32, tag="eyb")
            nc.vector.tensor_scalar_mul(out=eyb, in0=sums, scalar1=1.0 / L)
            ey = eyb[:, 0:1]
            ey2 = eyb[:, 1:2]
            yd2 = sb.tile([64, L], f32, tag="yd2")
            nc.vector.tensor_scalar(
                out=yd2, in0=y, scalar1=ey, scalar2=2.0, op0=OP.subtract, op1=OP.mult
            )

            # --- layer 1 (f32r, full-width) ---
            h_lv = ps.tile([128, L], f32, tag="hlv")
            h_mu = ps.tile([128, L], f32, tag="hmu")
            nc.tensor.matmul(h_lv, w1lvT_a, xa, start=True, stop=False)
            nc.tensor.matmul(h_mu, w1muT_a, xa, start=True, stop=False)
            nc.tensor.matmul(h_lv, w1lvT_b, xb, start=False, stop=True)
            nc.tensor.matmul(h_mu, w1muT_b, xb, start=False, stop=True)

            # --- layer 2 (bf16) + tail, chunked over L halves ---
            h_lv_s = sb.tile([128, L], bf16, tag="hlvs")
            h_mu_s = sb.tile([128, L], bf16, tag="hmus")
            acc4 = sb.tile([64, 4], f32, tag="acc4")
            lv_nbs = []
            mu_nbs = []
            for c in range(2):
                cs = slice(c * HC, (c + 1) * HC)
                # lv path first: it is one ACT stage deeper (tanh+exp)
                nc.scalar.activation(
                    out=h_lv_s[:, cs], in_=h_lv[:, cs], func=AF.Relu, bias=b1lv, scale=1.0
                )
                lv_nb = ps.tile([64, HC], f32, tag="lvnb")
                nc.tensor.matmul(lv_nb, w2lvT, h_lv_s[:, cs], start=True, stop=True)
                lv_nbs.append(lv_nb)
                # mu relu on DVE to keep ACT free for tanh/exp
                nc.vector.tensor_scalar(
                    out=h_mu_s[:, cs], in0=h_mu[:, cs], scalar1=b1mu, scalar2=0.0,
                    op0=OP.add, op1=OP.max,
                )
                mu_nb = ps.tile([64, HC], f32, tag="munb")
                nc.tensor.matmul(mu_nb, w2muT, h_mu_s[:, cs], start=True, stop=True)
                mu_nbs.append(mu_nb)

            # sum(iv) per half comes free from exp's accumulator; the ey2
            # cross-term collapses later via one dot-product matmul:
            #   sum((ysq-ey2)*iv) = sum(ysq*iv) - sum_d ey2[d]*sum_l iv[d,l]
            siv = sb.tile([64, 2], f32, tag="siv")
            for c in range(2):
                cs = slice(c * HC, (c + 1) * HC)
                t1 = sb.tile([64, HC], f32, tag="t1")
                nc.scalar.activation(
                    out=t1, in_=lv_nbs[c], func=AF.Tanh, bias=b2lv, scale=1.0
                )
                iv = sb.tile([64, HC], f32, tag="iv")
                nc.scalar.activation(
                    out=iv, in_=t1, func=AF.Exp, scale=-1.0,
                    accum_out=siv[:, c : c + 1],
                )
                z = sb.tile([64, HC], f32, tag="z")
                nc.vector.tensor_mul(z, yd2[:, cs], iv)
                # finA1 = sum(ysq * iv)
                scrA = sb.tile([64, HC], f32, tag="scrA")
                nc.vector.scalar_tensor_tensor(
                    out=scrA, in0=ysq[:, cs], scalar=1.0, in1=iv,
                    op0=OP.mult, op1=OP.mult, accum_out=acc4[:, 2 * c : 2 * c + 1],
                )
                # finB = sum((mu_nb + b2mu) * z)
                scrB = sb.tile([64, HC], f32, tag="scrB")
                nc.vector.scalar_tensor_tensor(
                    out=scrB, in0=mu_nbs[c], scalar=b2mu, in1=z,
                    op0=OP.add, op1=OP.mult, accum_out=acc4[:, 2 * c + 1 : 2 * c + 2],
                )

            acc_ps = ps.tile([4, 1], f32, tag="accps")
            nc.tensor.matmul(acc_ps, acc4, ones, start=True, stop=True)
            civ_ps = ps.tile([2, 1], f32, tag="civps")
            nc.tensor.matmul(civ_ps, siv, ey2, start=True, stop=True)
            acc_sb = sb.tile([4, 2], f32, tag="accsb")
            nc.scalar.copy(acc_sb[:, 0:1], acc_ps)
            nc.scalar.copy(acc_sb[0:2, 1:2], civ_ps)
            nc.sync.dma_start(out=acc_d[:, :], in_=acc_sb, single_packet=True)

    nc.compile()
    return nc


def pack_inputs(inputs: dict) -> list[dict]:
    import ml_dtypes

    x = np.ascontiguousarray(np.asarray(inputs["x_samples"], dtype=np.float32))
    y = np.ascontiguousarray(np.asarray(inputs["y_samples"], dtype=np.float32))
    mu_W1 = np.asarray(inputs["mu_W1"], dtype=np.float32)
    mu_b1 = np.asarray(inputs["mu_b1"], dtype=np.float32)
    mu_W2 = np.asarray(inputs["mu_W2"], dtype=np.float32)
    mu_b2 = np.asarray(inputs["mu_b2"], dtype=np.float32)
    lv_W1 = np.asarray(inputs["lv_W1"], dtype=np.float32)
    lv_b1 = np.asarray(inputs["lv_b1"], dtype=np.float32)
    lv_W2 = np.asarray(inputs["lv_W2"], dtype=np.float32)
    lv_b2 = np.asarray(inputs["lv_b2"], dtype=np.float32)

    w1 = np.zeros((128, 512), np.float32)
    w1muT = mu_W1.T  # [192, 128]
    w1lvT = lv_W1.T
    w1[:, 0:128] = w1muT[0:128]
    w1[:, 128:256] = w1lvT[0:128]
    w1[64:128, 256:384] = w1muT[128:192]
    w1[64:128, 384:512] = w1lvT[128:192]
    w2 = np.concatenate([mu_W2.T, lv_W2.T], axis=1).astype(ml_dtypes.bfloat16)
    b1 = np.ascontiguousarray(np.stack([mu_b1, lv_b1], axis=1))  # [128, 2]

    in_maps = []
    for b in range(NCORES):
        yb = np.zeros((64, YBC), np.float32)
        yb[:, 0:512] = y[b]
        yb[:, 512] = mu_b2
        yb[:, 513] = lv_b2
        in_maps.append(
            {
                "w1": w1,
                "w2": np.ascontiguousarray(w2),
                "xa": np.ascontiguousarray(x[b, 0:128]),
                "xb": np.ascontiguousarray(x[b, 128:192]),
                "yb": yb,
                "b1": b1,
            }
        )
    return in_maps


def kernel(**inputs) -> np.ndarray:
    from concourse.bass_utils import run_bass_kernel_spmd

    if "nc" not in _CACHE:
        _CACHE["nc"] = build_nc(debug=False)
    nc = _CACHE["nc"]

    in_maps = pack_inputs(inputs)
    res = run_bass_kernel_spmd(nc, in_maps, core_ids=list(range(NCORES)))
    tot = 0.0
    for r in res.results:
        a = r["acc"].astype(np.float64)  # [4, 2]
        tot += (a[0, 0] + a[2, 0]) - (a[1, 0] + a[3, 0]) - (a[0, 1] + a[1, 1])
    loss = -0.5 * tot / (B * L)
    return np.array(loss, dtype=np.float32)
